# revision 23
# baseline (speedup 1.0000x reference)
"""Trainium2 Bass kernel for nn_Attention (b=8, c=256, heads=4, dh=32, n=48*48).

Sharding: batch across 8 cores (attention independent per batch item);
qkv/out projection weights replicated (host pre-transposed/cast to fp16).

Per-core plan (one batch item, x_b [256, 2304]):
  1. QK projection      q,k [128(h*d), n] fp16                    (PE)
  2. V^T projection     vaug fp16 [vT_h (32) | ones (32)] tiles    (PE)
  3. i-blocks (4x512+256) outer, head-pairs inner; per j-tile:
     scores S^T[j,i] for both heads (row-tiled strips 32h) into a
     [128, 2, 512] PSUM tile (each head's matmul owns a bank)      (PE)
     -> one exp per j-tile on ScalarE covering both heads
        (scale=dh^-0.5 via the ACT affine; no max subtraction:
        scores ~ N(0,1) so fp32 exp is safe)                       (ACT)
     -> PV immediately (col-tiled across the two heads) with M=64
        stationary [vT|ones]: rows +0:32 = O^T, +32:64 = softmax
        denominator; accumulated over j-tiles in one PSUM bank     (PE)
  4. normalize per block: DVE reciprocal_approx_fast + multiply    (DVE)
  5. per-block y = w_out(f32r) @ onorm + b_out -> DMA out          (PE/DVE)

The exp stream (4*2304^2 elems/core @ 1 elem/cycle/lane @~1.2GHz,
~185us) is the roofline; everything else hides under it. Non-critical
projections (vT, q tiles 1-4) are emitted after the main loop so the
Tile scheduler runs them in PE idle gaps instead of delaying the first
exp.
"""

import sys

if "/opt/trn_rl_repo" not in sys.path:
    sys.path.insert(0, "/opt/trn_rl_repo")

import numpy as np

import concourse.bacc as bacc
import concourse.tile as tile
from concourse import mybir
from concourse.bass_utils import run_bass_kernel_spmd

HEADS = 4
DH = 32
HID = HEADS * DH          # 128
C = 256                   # channels
N = 48 * 48               # 2304 tokens
SCALE = DH ** -0.5
F32 = mybir.dt.float32
F32R = mybir.dt.float32r
F16 = mybir.dt.float16

NJT = N // 128            # 18 j-tiles
BLOCKS = [(0, 512), (512, 512), (1024, 512), (1536, 512), (2048, 256)]


def _kernel_body(tc, xd, wqkd, wvd, woutd, biasd, yd):
    nc = tc.nc
    import contextlib

    with contextlib.ExitStack() as stack:
        const = stack.enter_context(tc.tile_pool(name="const", bufs=1))
        qkp = stack.enter_context(tc.tile_pool(name="qkp", bufs=1))
        vap = stack.enter_context(tc.tile_pool(name="vap", bufs=1))
        onp = stack.enter_context(tc.tile_pool(name="onp", bufs=1))
        xp = stack.enter_context(tc.tile_pool(name="xp", bufs=1))
        ppsum = stack.enter_context(tc.tile_pool(name="ppsum", bufs=2, space="PSUM"))
        ystp = stack.enter_context(tc.tile_pool(name="ystp", bufs=3))

        xs = xp.tile([128, 2, N], F16, name="xs")
        wqk = const.tile([128, 2, 2 * HID], F16, name="wqk")
        wv = const.tile([128, 2, HID], F16, name="wv")
        wout = const.tile([128, C], F16, name="wout")
        bias = const.tile([128, 2], F32, name="bias")
        # weight DMAs first (k-projection stationarys), then x column-split
        # so the first q/k tiles land as early as possible
        nc.sync.dma_start(out=wqk[:, 0, :], in_=wqkd[0:128, :])
        nc.sync.dma_start(out=wqk[:, 1, :], in_=wqkd[128:256, :])
        for cc in range(2):
            nc.sync.dma_start(
                out=xs[:, cc, 0:512],
                in_=xd[128 * cc : 128 * cc + 128, 0:512],
            )
        nc.sync.dma_start(out=wv[:, 0, :], in_=wvd[0:128, :])
        nc.sync.dma_start(out=wv[:, 1, :], in_=wvd[128:256, :])
        for cc in range(2):
            nc.sync.dma_start(
                out=xs[:, cc, 512:N],
                in_=xd[128 * cc : 128 * cc + 128, 512:N],
            )
        nc.sync.dma_start(out=wout[:, :], in_=woutd[:, :])
        nc.sync.dma_start(out=bias[:, :], in_=biasd[:, :])

        q = qkp.tile([128, N], F16, name="q")
        k = qkp.tile([128, N], F16, name="k")
        vaug = vap.tile([128, NJT, HEADS, 2 * DH], F16, name="vaug")
        onorm = onp.tile([128, N], F16, name="onorm")

        nc.vector.memset(vaug[:, :, :, DH : 2 * DH], 1.0)

        # dummy exp: forces the ACT table load during the DMA wait
        warm = const.tile([1, 1], F32, name="warm")
        nc.vector.memset(warm[:, :], 0.0)
        nc.scalar.activation(warm[:, :], warm[:, :], mybir.ActivationFunctionType.Exp)

        def qk_proj(m, off, w):
            # out[m-rows, i] = sum_c wqk[c, m] * x[c, i]
            dst = q if m == 0 else k
            pt = ppsum.tile([128, 512], F32, name="pt", tag="pt")
            for cc in range(2):
                nc.tensor.matmul(
                    pt[:, 0:w],
                    wqk[:, cc, 128 * m : 128 * m + 128],
                    xs[:, cc, off : off + w],
                    start=(cc == 0),
                    stop=(cc == 1),
                )
            nc.vector.tensor_copy(dst[:, off : off + w], pt[:, 0:w])

        # critical path to the first exp: k tile0 + q tile0 (k tiles 1-4 are
        # emitted after the main loop as dependency-gated gap fillers)
        qk_proj(1, 0, 512)
        qk_proj(0, 0, 512)

        def out_proj(goff, w):
            for m in range(2):
                yp = ppsum.tile([128, 512], F32, name="yp", tag="pt")
                nc.tensor.matmul(
                    yp[:, 0:w],
                    wout[:, 128 * m : 128 * m + 128],
                    onorm[:, goff : goff + w],
                    start=True,
                    stop=True,
                )
                yst = ystp.tile([128, 512], F32, name="yst", tag="yst")
                nc.vector.tensor_scalar_add(yst[:, 0:w], yp[:, 0:w], bias[:, m : m + 1])
                nc.sync.dma_start(
                    out=yd[128 * m : 128 * m + 128, goff : goff + w],
                    in_=yst[:, 0:w],
                )

        # ---------------- main attention loop ----------------
        with (
            tc.tile_pool(name="esp", bufs=14) as esp,
            tc.tile_pool(name="scp", bufs=2, space="PSUM") as scp,
            tc.tile_pool(name="accp", bufs=2, space="PSUM") as accp,
            tc.tile_pool(name="recp", bufs=4) as recp,
        ):
            for bi, (goff, w) in enumerate(BLOCKS):
                jtg = 2 if w <= 256 else 1   # group jts per exp for narrow blocks
                for hp in range(2):
                    h0 = 2 * hp
                    acc = accp.tile([128, 512], F32, name="acc", tag="acc")
                    for jt0 in range(0, NJT, jtg):
                        # h-major layout: each head's chunks stay in its own
                        # bank so the row-tiled matmul pair never shares one
                        sc = scp.tile([128, 2, jtg, 512 // jtg], F32, name="sc", tag="sc")
                        for jl in range(jtg):
                            jt = jt0 + jl
                            for hh in range(2):
                                h = h0 + hh
                                nc.tensor.matmul(
                                    sc[:, hh, jl, 0:w],
                                    k[32 * h : 32 * h + 32, 128 * jt : 128 * jt + 128],
                                    q[32 * h : 32 * h + 32, goff : goff + w],
                                    start=True,
                                    stop=True,
                                    tile_position=(32 * h, 0),
                                )
                        es = esp.tile([128, 2, jtg, 512 // jtg], F16, name="es", tag="es")
                        if w * jtg == 512:
                            nc.scalar.activation(
                                es[:, :, :, :],
                                sc[:, :, :, :],
                                mybir.ActivationFunctionType.Exp,
                                scale=SCALE,
                            )
                        else:
                            nc.scalar.activation(
                                es[:, :, 0, 0:w],
                                sc[:, :, 0, 0:w],
                                mybir.ActivationFunctionType.Exp,
                                scale=SCALE,
                            )
                        for jl in range(jtg):
                            jt = jt0 + jl
                            for hh in range(2):
                                nc.tensor.matmul(
                                    acc[64 * hh : 64 * hh + 64, 0:w],
                                    vaug[:, jt, h0 + hh, :],
                                    es[:, hh, jl, 0:w],
                                    start=(jt == 0),
                                    stop=(jt == NJT - 1),
                                    tile_position=(0, 64 * hh),
                                )
                    # stage the next block's q tile before this block's
                    # normalize/bias DVE backlog (it gates the next block's
                    # score matmuls via its DVE cast)
                    if hp == 0 and bi + 1 < len(BLOCKS):
                        noff, nw = BLOCKS[bi + 1]
                        qk_proj(0, noff, nw)
                    # previous block's output projection, deferred here to
                    # keep it off the congested block boundary
                    if hp == 0 and bi > 0:
                        out_proj(*BLOCKS[bi - 1])
                    # normalize: O * (1/denom); denom dup'd across 32 rows
                    for hh in range(2):
                        h = h0 + hh
                        p0 = 64 * hh
                        rec = recp.tile([32, 512], F32, name="rec", tag="rec")
                        den = recp.tile([32, 512], F32, name="den", tag="den")
                        dcp = nc.scalar.copy if (bi == len(BLOCKS) - 1 and hp == 1) else nc.vector.tensor_copy
                        dcp(den[:, 0:w], acc[p0 + 32 : p0 + 64, 0:w])
                        nc.vector.reciprocal_approx_fast(rec[:, 0:w], den[:, 0:w])
                        nc.vector.tensor_mul(
                            onorm[32 * h : 32 * h + 32, goff : goff + w],
                            acc[p0 : p0 + 32, 0:w],
                            rec[:, 0:w],
                        )
            out_proj(*BLOCKS[-1])

            # ---- gap fillers: emitted last so the scheduler slots them ----
            # ---- into PE idle time instead of delaying the first exp  ----
            for off, w in BLOCKS[1:]:
                qk_proj(1, off, w)
            for nt in range(NJT):
                pv = ppsum.tile([128, HID], F32, name="pv", tag="pt")
                for cc in range(2):
                    nc.tensor.matmul(
                        pv[:, :],
                        xs[:, cc, 128 * nt : 128 * nt + 128],
                        wv[:, cc, :],
                        start=(cc == 0),
                        stop=(cc == 1),
                    )
                nc.vector.tensor_copy(
                    vaug[:, nt, :, 0:DH],
                    pv.rearrange("p (h d) -> p h d", h=HEADS),
                )



_CACHE = {}


def _build():
    if "nc" in _CACHE:
        return _CACHE["nc"]
    nc = bacc.Bacc("TRN2", target_bir_lowering=False, debug=False)
    xd = nc.dram_tensor("x", [C, N], F16, kind="ExternalInput")
    wqkd = nc.dram_tensor("wqk_t", [C, 2 * HID], F16, kind="ExternalInput")
    wvd = nc.dram_tensor("wv_t", [C, HID], F16, kind="ExternalInput")
    woutd = nc.dram_tensor("wout_t", [HID, C], F16, kind="ExternalInput")
    biasd = nc.dram_tensor("bias2", [128, 2], F32, kind="ExternalInput")
    yd = nc.dram_tensor("y", [C, N], F32, kind="ExternalOutput")
    with tile.TileContext(nc) as tc:
        _kernel_body(tc, xd.ap(), wqkd.ap(), wvd.ap(), woutd.ap(), biasd.ap(), yd.ap())
    nc.compile()
    _CACHE["nc"] = nc
    return nc


def _make_in_maps(x, w_qkv, w_out, b_out):
    x = np.asarray(x, dtype=np.float32).reshape(8, C, N)
    x16 = np.ascontiguousarray(x.astype(np.float16))
    w_qkv = np.asarray(w_qkv, dtype=np.float32)
    w_out = np.asarray(w_out, dtype=np.float32)
    b_out = np.asarray(b_out, dtype=np.float32)
    wqk_t = np.ascontiguousarray(w_qkv[0 : 2 * HID].T.astype(np.float16))       # [256, 256]
    wv_t = np.ascontiguousarray(w_qkv[2 * HID : 3 * HID].T.astype(np.float16))  # [256, 128]
    wout_t = np.ascontiguousarray(w_out.T.astype(np.float16))   # [128, 256]
    bias2 = np.ascontiguousarray(b_out.reshape(2, 128).T)       # [128, 2]
    return [
        {"x": x16[b], "wqk_t": wqk_t, "wv_t": wv_t, "wout_t": wout_t, "bias2": bias2}
        for b in range(8)
    ]


def kernel(x, w_qkv, w_out, b_out, _trace=False):
    nc = _build()
    in_maps = _make_in_maps(x, w_qkv, w_out, b_out)
    res = run_bass_kernel_spmd(nc, in_maps, list(range(8)), trace=_trace)
    y = np.stack([res.results[b]["y"] for b in range(8)], axis=0)
    out = y.reshape(8, C, 48, 48).astype(np.float32)
    if _trace:
        _CACHE["last_exec_time_ns"] = res.exec_time_ns
        _CACHE["last_results"] = res
    return out


# revision 25
# speedup vs baseline: 1.1049x; 1.1049x over previous
"""Trainium2 Bass kernel for nn_Attention (b=8, c=256, heads=4, dh=32, n=48*48).

Sharding: batch across 8 cores (attention independent per batch item);
qkv/out projection weights replicated (host pre-transposed/cast to fp16).

Per-core plan (one batch item, x_b [256, 2304]):
  1. QK projection      q,k [128(h*d), n] fp16                    (PE)
  2. V^T projection     vaug fp16 [vT_h (32) | ones (32)] tiles    (PE)
  3. i-blocks (4x512+256) outer, head-pairs inner; per j-tile:
     scores S^T[j,i] for both heads (row-tiled strips 32h) into a
     [128, 2, 512] PSUM tile (each head's matmul owns a bank)      (PE)
     -> one exp per j-tile on ScalarE covering both heads
        (scale=dh^-0.5 via the ACT affine; no max subtraction:
        scores ~ N(0,1) so fp32 exp is safe)                       (ACT)
     -> PV immediately (col-tiled across the two heads) with M=64
        stationary [vT|ones]: rows +0:32 = O^T, +32:64 = softmax
        denominator; accumulated over j-tiles in one PSUM bank     (PE)
  4. normalize per block: DVE reciprocal_approx_fast + multiply    (DVE)
  5. per-block y = w_out(f32r) @ onorm + b_out -> DMA out          (PE/DVE)

The exp stream (4*2304^2 elems/core @ 1 elem/cycle/lane @~1.2GHz,
~185us) is the roofline; everything else hides under it. Non-critical
projections (vT, q tiles 1-4) are emitted after the main loop so the
Tile scheduler runs them in PE idle gaps instead of delaying the first
exp.
"""

import sys

if "/opt/trn_rl_repo" not in sys.path:
    sys.path.insert(0, "/opt/trn_rl_repo")

import numpy as np

import concourse.bacc as bacc
import concourse.tile as tile
from concourse import mybir
from concourse.bass_utils import run_bass_kernel_spmd

HEADS = 4
DH = 32
HID = HEADS * DH          # 128
C = 256                   # channels
N = 48 * 48               # 2304 tokens
SCALE = DH ** -0.5
F32 = mybir.dt.float32
F32R = mybir.dt.float32r
F16 = mybir.dt.float16

NJT = N // 128            # 18 j-tiles
BLOCKS = [(0, 512), (512, 512), (1024, 512), (1536, 512), (2048, 256)]


def _kernel_body(tc, xd, wqkd, wvd, woutd, biasd, yd):
    nc = tc.nc
    import contextlib

    with contextlib.ExitStack() as stack:
        const = stack.enter_context(tc.tile_pool(name="const", bufs=1))
        qkp = stack.enter_context(tc.tile_pool(name="qkp", bufs=1))
        vap = stack.enter_context(tc.tile_pool(name="vap", bufs=1))
        onp = stack.enter_context(tc.tile_pool(name="onp", bufs=1))
        xp = stack.enter_context(tc.tile_pool(name="xp", bufs=1))
        scp = stack.enter_context(tc.tile_pool(name="scp", bufs=3, space="PSUM"))
        ystp = stack.enter_context(tc.tile_pool(name="ystp", bufs=3))

        xs = xp.tile([128, 2, N], F16, name="xs")
        wqk = const.tile([128, 2, 2 * HID], F16, name="wqk")
        wv = const.tile([128, 2, HID], F16, name="wv")
        wout = const.tile([128, C], F16, name="wout")
        bias = const.tile([128, 2], F32, name="bias")
        # first x columns + wqk first: they gate the first q/k tiles
        for cc in range(2):
            nc.sync.dma_start(
                out=xs[:, cc, 0:512],
                in_=xd[128 * cc : 128 * cc + 128, 0:512],
            )
        nc.sync.dma_start(out=wqk[:, 0, :], in_=wqkd[0:128, :])
        nc.sync.dma_start(out=wqk[:, 1, :], in_=wqkd[128:256, :])
        nc.sync.dma_start(out=wv[:, 0, :], in_=wvd[0:128, :])
        nc.sync.dma_start(out=wv[:, 1, :], in_=wvd[128:256, :])
        for cc in range(2):
            nc.sync.dma_start(
                out=xs[:, cc, 512:N],
                in_=xd[128 * cc : 128 * cc + 128, 512:N],
            )
        nc.sync.dma_start(out=wout[:, :], in_=woutd[:, :])
        nc.sync.dma_start(out=bias[:, :], in_=biasd[:, :])

        q = qkp.tile([128, N], F16, name="q")
        k = qkp.tile([128, N], F16, name="k")
        vaug = vap.tile([128, NJT, HEADS, 2 * DH], F16, name="vaug")
        onorm = onp.tile([128, N], F16, name="onorm")

        nc.gpsimd.memset(vaug[:, :, :, DH : 2 * DH], 1.0)

        # dummy exp: forces the ACT table load during the DMA wait
        warm = const.tile([1, 1], F32, name="warm")
        nc.gpsimd.memset(warm[:, :], 0.0)
        nc.scalar.activation(warm[:, :], warm[:, :], mybir.ActivationFunctionType.Exp)

        def qk_proj(m, off, w):
            # out[m-rows, i] = sum_c wqk[c, m] * x[c, i]
            dst = q if m == 0 else k
            pt = scp.tile([128, 2, 1, 512], F32, name="pt", tag="sc")
            for cc in range(2):
                nc.tensor.matmul(
                    pt[:, 0, 0, 0:w],
                    wqk[:, cc, 128 * m : 128 * m + 128],
                    xs[:, cc, off : off + w],
                    start=(cc == 0),
                    stop=(cc == 1),
                )
            nc.vector.tensor_copy(dst[:, off : off + w], pt[:, 0, 0, 0:w])

        # critical path to the first exp: k tile0 + q tile0 (k tiles 1-4 are
        # emitted after the main loop as dependency-gated gap fillers)
        qk_proj(1, 0, 512)
        qk_proj(0, 0, 512)

        def out_proj(goff, w):
            for m in range(2):
                yp = scp.tile([128, 2, 1, 512], F32, name="yp", tag="sc")
                nc.tensor.matmul(
                    yp[:, 0, 0, 0:w],
                    wout[:, 128 * m : 128 * m + 128],
                    onorm[:, goff : goff + w],
                    start=True,
                    stop=True,
                )
                yst = ystp.tile([128, 512], F32, name="yst", tag="yst")
                nc.vector.tensor_scalar_add(yst[:, 0:w], yp[:, 0, 0, 0:w], bias[:, m : m + 1])
                nc.sync.dma_start(
                    out=yd[128 * m : 128 * m + 128, goff : goff + w],
                    in_=yst[:, 0:w],
                )

        # ---------------- main attention loop ----------------
        with (
            tc.tile_pool(name="esp", bufs=14) as esp,
            tc.tile_pool(name="accp", bufs=2, space="PSUM") as accp,
            tc.tile_pool(name="recp", bufs=4) as recp,
        ):
            for bi, (goff, w) in enumerate(BLOCKS):
                jtg = 2 if w <= 256 else 1   # group jts per exp for narrow blocks
                for hp in range(2):
                    h0 = 2 * hp
                    acc = accp.tile([128, 512], F32, name="acc", tag="acc")
                    for jt0 in range(0, NJT, jtg):
                        # h-major layout: each head's chunks stay in its own
                        # bank so the row-tiled matmul pair never shares one
                        sc = scp.tile([128, 2, jtg, 512 // jtg], F32, name="sc", tag="sc")
                        for jl in range(jtg):
                            jt = jt0 + jl
                            for hh in range(2):
                                h = h0 + hh
                                nc.tensor.matmul(
                                    sc[:, hh, jl, 0:w],
                                    k[32 * h : 32 * h + 32, 128 * jt : 128 * jt + 128],
                                    q[32 * h : 32 * h + 32, goff : goff + w],
                                    start=True,
                                    stop=True,
                                    tile_position=(32 * h, 0),
                                )
                        es = esp.tile([128, 2, jtg, 512 // jtg], F16, name="es", tag="es")
                        if w * jtg == 512:
                            nc.scalar.activation(
                                es[:, :, :, :],
                                sc[:, :, :, :],
                                mybir.ActivationFunctionType.Exp,
                                scale=SCALE,
                            )
                        else:
                            nc.scalar.activation(
                                es[:, :, 0, 0:w],
                                sc[:, :, 0, 0:w],
                                mybir.ActivationFunctionType.Exp,
                                scale=SCALE,
                            )
                        for jl in range(jtg):
                            jt = jt0 + jl
                            for hh in range(2):
                                nc.tensor.matmul(
                                    acc[64 * hh : 64 * hh + 64, 0:w],
                                    vaug[:, jt, h0 + hh, :],
                                    es[:, hh, jl, 0:w],
                                    start=(jt == 0),
                                    stop=(jt == NJT - 1),
                                    tile_position=(0, 64 * hh),
                                )
                    # stage the next block's q tile before this block's
                    # normalize/bias DVE backlog (it gates the next block's
                    # score matmuls via its DVE cast)
                    if hp == 0 and bi + 1 < len(BLOCKS):
                        noff, nw = BLOCKS[bi + 1]
                        qk_proj(0, noff, nw)
                    # previous block's output projection, deferred here to
                    # keep it off the congested block boundary
                    if hp == 0 and bi > 0:
                        out_proj(*BLOCKS[bi - 1])
                    # normalize: O * (1/denom); denom dup'd across 32 rows
                    for hh in range(2):
                        h = h0 + hh
                        p0 = 64 * hh
                        rec = recp.tile([32, 512], F32, name="rec", tag="rec")
                        den = recp.tile([32, 512], F32, name="den", tag="den")
                        dcp = nc.scalar.copy if (bi == len(BLOCKS) - 1 and hp == 1) else nc.vector.tensor_copy
                        dcp(den[:, 0:w], acc[p0 + 32 : p0 + 64, 0:w])
                        nc.vector.reciprocal_approx_fast(rec[:, 0:w], den[:, 0:w])
                        nc.vector.tensor_mul(
                            onorm[32 * h : 32 * h + 32, goff : goff + w],
                            acc[p0 : p0 + 32, 0:w],
                            rec[:, 0:w],
                        )
            out_proj(*BLOCKS[-1])

            # ---- gap fillers: emitted last so the scheduler slots them ----
            # ---- into PE idle time instead of delaying the first exp  ----
            for off, w in BLOCKS[1:]:
                qk_proj(1, off, w)
            for nt in range(NJT):
                pv = scp.tile([128, 2, 1, 512], F32, name="pv", tag="sc")
                for cc in range(2):
                    nc.tensor.matmul(
                        pv[:, 0, 0, 0:HID],
                        xs[:, cc, 128 * nt : 128 * nt + 128],
                        wv[:, cc, :],
                        start=(cc == 0),
                        stop=(cc == 1),
                    )
                nc.vector.tensor_copy(
                    vaug[:, nt, :, 0:DH],
                    pv[:, 0, 0, 0:HID].rearrange("p (h d) -> p h d", h=HEADS),
                )



_CACHE = {}


def _build():
    if "nc" in _CACHE:
        return _CACHE["nc"]
    nc = bacc.Bacc("TRN2", target_bir_lowering=False, debug=False)
    xd = nc.dram_tensor("x", [C, N], F16, kind="ExternalInput")
    wqkd = nc.dram_tensor("wqk_t", [C, 2 * HID], F16, kind="ExternalInput")
    wvd = nc.dram_tensor("wv_t", [C, HID], F16, kind="ExternalInput")
    woutd = nc.dram_tensor("wout_t", [HID, C], F16, kind="ExternalInput")
    biasd = nc.dram_tensor("bias2", [128, 2], F32, kind="ExternalInput")
    yd = nc.dram_tensor("y", [C, N], F32, kind="ExternalOutput")
    with tile.TileContext(nc) as tc:
        _kernel_body(tc, xd.ap(), wqkd.ap(), wvd.ap(), woutd.ap(), biasd.ap(), yd.ap())
    nc.compile()
    _CACHE["nc"] = nc
    return nc


def _make_in_maps(x, w_qkv, w_out, b_out):
    x = np.asarray(x, dtype=np.float32).reshape(8, C, N)
    x16 = np.ascontiguousarray(x.astype(np.float16))
    w_qkv = np.asarray(w_qkv, dtype=np.float32)
    w_out = np.asarray(w_out, dtype=np.float32)
    b_out = np.asarray(b_out, dtype=np.float32)
    wqk_t = np.ascontiguousarray(w_qkv[0 : 2 * HID].T.astype(np.float16))       # [256, 256]
    wv_t = np.ascontiguousarray(w_qkv[2 * HID : 3 * HID].T.astype(np.float16))  # [256, 128]
    wout_t = np.ascontiguousarray(w_out.T.astype(np.float16))   # [128, 256]
    bias2 = np.ascontiguousarray(b_out.reshape(2, 128).T)       # [128, 2]
    return [
        {"x": x16[b], "wqk_t": wqk_t, "wv_t": wv_t, "wout_t": wout_t, "bias2": bias2}
        for b in range(8)
    ]


def kernel(x, w_qkv, w_out, b_out, _trace=False):
    nc = _build()
    in_maps = _make_in_maps(x, w_qkv, w_out, b_out)
    res = run_bass_kernel_spmd(nc, in_maps, list(range(8)), trace=_trace)
    y = np.stack([res.results[b]["y"] for b in range(8)], axis=0)
    out = y.reshape(8, C, 48, 48).astype(np.float32)
    if _trace:
        _CACHE["last_exec_time_ns"] = res.exec_time_ns
        _CACHE["last_results"] = res
    return out
